# revision 31
# baseline (speedup 1.0000x reference)
"""Trainium2 Bass kernel for nn_Attention_57432302682539.

Reference computation (B=32, S=4096, D=256, H=256):
    inp = x @ W_in.T + b_in                                  # [B, H]
    branch_i: ctx = einsum('bsd,hd->bhs', context, Wc_i) + bc_i
              att_i = einsum('h,bhs->bs', V_i, tanh(inp[:,:,None] + ctx))
    att = concat(att_0..3, axis=1)                           # [B, 4S]
    att = 10*tanh(att)  (mask is all ones -> where() is identity)
    out = softmax(att, axis=0)                               # over batch

Sharding: BRANCH x S-HALF.  Core c handles branch br=c>>1 and s-columns
[sh*2048, sh*2048+2048) with sh=c&1, for ALL 32 batches.  The dim-0
(batch) softmax acts per output column (br, s), so with every batch
resident the softmax is entirely core-local -- no collective.

Compared to the previous S-only shard this grows the per-(b,ht) tanh
tile from [128,512] to [128,2048], quartering the ACT instruction count
(the scalar engine, at 1 elem/cycle/lane with ~200ns/instr overhead, is
the bottleneck engine together with PE).

Per-core pipeline, per batch b (32 iterations):
  - DMA ctxT[b] [2dt,128,2048] fp16 (prefetched, triple-buffered)
  - PE: wcT.T @ ctx -> PSUM units [128h,1024s] (K=256 via 2 accum steps)
  - ACT: tanh with per-partition bias (inp[b]+bc) -> SBUF fp16 T tiles
  - DVE: s_t = V_ht0*T0 + V_ht1*T1  (per-partition scalars)
  - PE: ones-window matmul routes partition-sum of s_t chunk c into row
    c*32+b of a single persistent PSUM accumulator [128, 512]
  - final: exp(10*tanh(att)) on ACT (fp16), batch-group sums via a
    2^-12-scaled fp16 select matmul, fp16 DVE reciprocal (the scale keeps
    it in fp16 normal range), K=4 select matmul to broadcast 1/den back
    to 128 partitions, DVE mul, out-DMAs split across both HWDGE rings.

PSUM budget (8 banks): 3 x 2-bank tanh units + 1 att accumulator +
1 misc (inp matmul, then softmax denominator).
"""

import os
import numpy as np

B, S, D, H = 32, 4096, 256, 256
NCORES = 8
NBR = 4                   # branches
SH = S // 2               # 2048 s-positions per core (one half)
HT = 2                    # h tiles of 128
DTILES = 2                # d tiles of 128
P = 128
NCHUNK = 4                # 512-col chunks of SH; att row = chunk*32 + b

_CACHE = {}


def _build_nc(dt_name: str, repeat: int = 1, loop_n: int = 0, variant: str = "full"):
    """Build the Bass module. dt_name in ('float16', 'bfloat16').

    repeat>1 unrolls the whole computation N times inside the NEFF; loop_n>0
    instead wraps it in a hardware For_i loop with that trip count (both for
    on-device timing via wall-clock differencing); the result is unchanged.
    """
    import concourse.bass as bass
    import concourse.mybir as mybir
    import concourse.tile as tile
    from concourse import bacc

    DT = getattr(mybir.dt, dt_name)
    F32 = mybir.dt.float32
    AF = mybir.ActivationFunctionType

    nc = bacc.Bacc(
        trn_type="TRN2",
        use_seq_codegen=os.environ.get("KERNEL_SEQ_CODEGEN", "0") == "1",
    )

    # Per-core external inputs (host-preprocessed, branch/s-half specific).
    ctxT = nc.dram_tensor("ctxT", [B, DTILES, P, SH], DT, kind="ExternalInput")
    wcT = nc.dram_tensor("wcT", [P, DTILES, HT, P], DT, kind="ExternalInput")
    ones_w = nc.dram_tensor("ones_w", [P, 256], DT, kind="ExternalInput")
    vcols = nc.dram_tensor("vcols", [P, HT], F32, kind="ExternalInput")
    winT = nc.dram_tensor("winT", [P, DTILES, HT, P], DT, kind="ExternalInput")
    xT = nc.dram_tensor("xT", [P, DTILES, B], DT, kind="ExternalInput")
    bcomb = nc.dram_tensor("bcomb", [P, HT], F32, kind="ExternalInput")
    sel = nc.dram_tensor("sel", [P, NCHUNK], DT, kind="ExternalInput")
    selb = nc.dram_tensor("selb", [NCHUNK, P], DT, kind="ExternalInput")
    out = nc.dram_tensor("out", [B, SH], F32, kind="ExternalOutput")

    from contextlib import ExitStack

    with tile.TileContext(nc) as tc, ExitStack() as ctx:
        const = ctx.enter_context(tc.tile_pool(name="const", bufs=1))
        ctxp = ctx.enter_context(
            tc.tile_pool(
                name="ctxp", bufs=int(os.environ.get("KERNEL_PF", "6")) + 1
            )
        )
        tanhp = ctx.enter_context(tc.tile_pool(name="tanhp", bufs=10))
        stp = ctx.enter_context(tc.tile_pool(name="stp", bufs=6))
        finalp = ctx.enter_context(tc.tile_pool(name="finalp", bufs=2))
        ps_tanh = ctx.enter_context(tc.tile_pool(name="ps_tanh", bufs=3, space="PSUM"))
        ps_att = ctx.enter_context(tc.tile_pool(name="ps_att", bufs=1, space="PSUM"))
        ps_misc = ctx.enter_context(tc.tile_pool(name="ps_misc", bufs=1, space="PSUM"))

        # ---- constants into SBUF ----
        wcT_sb = const.tile([P, DTILES, HT, P], DT)
        nc.sync.dma_start(out=wcT_sb, in_=wcT[:])
        ones_sb = const.tile([P, 256], DT)
        nc.sync.dma_start(out=ones_sb, in_=ones_w[:])
        vcols_sb = const.tile([P, HT], F32)
        nc.sync.dma_start(out=vcols_sb, in_=vcols[:])
        winT_sb = const.tile([P, DTILES, HT, P], DT)
        nc.sync.dma_start(out=winT_sb, in_=winT[:])
        xT_sb = const.tile([P, DTILES, B], DT)
        nc.sync.dma_start(out=xT_sb, in_=xT[:])
        bcomb_sb = const.tile([P, HT], F32)
        nc.sync.dma_start(out=bcomb_sb, in_=bcomb[:])
        sel_sb = const.tile([P, NCHUNK], DT)
        nc.sync.dma_start(out=sel_sb, in_=sel[:])
        selb_sb = const.tile([NCHUNK, P], DT)
        nc.sync.dma_start(out=selb_sb, in_=selb[:])

        do_act = variant not in ("mm_only",)
        do_dve = variant not in ("mm_only", "act_only")
        do_vsum = variant not in ("mm_only", "act_only", "no_vsum")
        do_final = do_vsum

        def emit_body():
            # ---- inp = x @ W_in.T (+ b_in + bc), h on partitions ----
            # bias_all[:, ht, b] = sum_d W_in[h,d] x[b,d] + b_in[h] + bc[h]
            bias_all = const.tile([P, HT, B], F32, tag="bias", name="bias_all")
            for ht in range(HT):
                ps_inp = ps_misc.tile([P, B], F32, tag="ps_misc", name="ps_inp")
                for dti in range(DTILES):
                    nc.tensor.matmul(
                        ps_inp[:],
                        lhsT=winT_sb[:, dti, ht],
                        rhs=xT_sb[:, dti],
                        start=(dti == 0),
                        stop=(dti == DTILES - 1),
                    )
                nc.scalar.add(bias_all[:, ht], ps_inp[:], bcomb_sb[:, ht : ht + 1])

            att_ps = ps_att.tile([P, 512], F32, tag="att", name="att_ps")

            # lagged work queues
            pend_dve = []   # (T0_half, T1_half, b, half)
            pend_mm = []    # (s_t half tile, b, half)
            dve_idx = [0]
            mm_idx = [0]
            n_halves = B * 2
            # halves between ACT and the DVE V-weighting / further halves
            # between DVE and the PE partition-sum
            DVE_LAG = int(os.environ.get("KERNEL_DVE_LAG", "2"))
            MM_LAG = int(os.environ.get("KERNEL_MM_LAG", "4"))

            def emit_dve():
                ip = dve_idx[0]
                T0, T1, b, half = pend_dve[ip]
                u_t = stp.tile([P, 1024], DT, tag="acc", name="u_t")
                nc.vector.tensor_scalar_mul(u_t[:], T0[:], vcols_sb[:, 0:1])
                w_t = stp.tile([P, 1024], DT, tag="acc", name="w_t")
                nc.vector.tensor_scalar_mul(w_t[:], T1[:], vcols_sb[:, 1:2])
                s_t = stp.tile([P, 1024], DT, tag="acc", name="s_t")
                nc.vector.tensor_add(s_t[:], u_t[:], w_t[:])
                pend_mm.append((s_t, b, half))
                dve_idx[0] += 1

            def emit_vsum():
                ip = mm_idx[0]
                s_t, b, half = pend_mm[ip]
                for uc in range(2):
                    c = half * 2 + uc
                    j = c * 32 + b
                    nc.tensor.matmul(
                        att_ps[:],
                        lhsT=ones_sb[:, 127 - j : 255 - j],
                        rhs=s_t[:, uc * 512 : (uc + 1) * 512],
                        start=(ip == 0 and uc == 0),
                        stop=(ip == n_halves - 1 and uc == 1),
                    )
                mm_idx[0] += 1

            PF = int(os.environ.get("KERNEL_PF", "6"))  # ctx DMA prefetch depth
            ctx_tiles = [None] * B

            def load_ctx(b):
                tp = ctxp.tile([P, DTILES, SH], DT, tag="ctx", name="ctxb")
                src = ctxT[b].rearrange("dt p s -> p dt s")
                nc.sync.dma_start(out=tp, in_=src)
                ctx_tiles[b] = tp

            # b=0 lands as two separate tiles (512 + 1536 s-cols) so the
            # first fill matmuls start after ~0.8us instead of ~3.2us;
            # chunk reads below are 512-aligned so the split is transparent.
            ctx0_parts = [None, None]

            def load_ctx0_split():
                pa = ctxp.tile([P, DTILES, 512], DT, tag="ctx0a", name="ctx0a")
                nc.sync.dma_start(
                    out=pa, in_=ctxT[0, :, :, 0:512].rearrange("dt p s -> p dt s")
                )
                pb = ctxp.tile([P, DTILES, SH - 512], DT, tag="ctx0b", name="ctx0b")
                nc.sync.dma_start(
                    out=pb, in_=ctxT[0, :, :, 512:SH].rearrange("dt p s -> p dt s")
                )
                ctx0_parts[0], ctx0_parts[1] = pa, pb

            def ctx_chunk(b, dti, s0):
                # [128, 512] chunk of batch b's context at s-offset s0
                if b == 0 or (variant == "no_dma"):
                    if s0 == 0:
                        return ctx0_parts[0][:, dti, 0:512]
                    return ctx0_parts[1][:, dti, s0 - 512 : s0]
                return ctx_tiles[b][:, dti, s0 : s0 + 512]

            if variant == "no_dma":
                load_ctx0_split()
            else:
                load_ctx0_split()
                for b in range(1, min(PF, B)):
                    load_ctx(b)

            for b in range(B):
                if variant != "no_dma" and b + PF < B:
                    load_ctx(b + PF)
                T_halves = [[None, None], [None, None]]  # [half][ht]
                for half in range(2):
                    for ht in range(HT):
                        ut = ps_tanh.tile([P, 1024], F32, tag="pt", name="ut")
                        for uc in range(2):
                            s0 = half * 1024 + uc * 512
                            for dti in range(DTILES):
                                nc.tensor.matmul(
                                    ut[:, uc * 512 : (uc + 1) * 512],
                                    lhsT=wcT_sb[:, dti, ht],
                                    rhs=ctx_chunk(b, dti, s0),
                                    start=(dti == 0),
                                    stop=(dti == DTILES - 1),
                                )
                        if do_act:
                            tt = tanhp.tile([P, 1024], DT, tag="tanh", name="tt")
                            nc.scalar.activation(
                                tt[:], ut[:], AF.Tanh, bias=bias_all[:, ht, b : b + 1]
                            )
                            T_halves[half][ht] = tt
                    if do_dve:
                        pend_dve.append(
                            (T_halves[half][0], T_halves[half][1], b, half)
                        )
                        ip = len(pend_dve) - 1
                        if ip >= DVE_LAG:
                            emit_dve()
                        if do_vsum and ip >= DVE_LAG + MM_LAG:
                            emit_vsum()
                if variant != "no_dma":
                    ctx_tiles[b] = None

            if do_dve:
                while dve_idx[0] < n_halves:
                    emit_dve()
            if do_vsum:
                while mm_idx[0] < n_halves:
                    emit_vsum()

            if not do_final:
                return

            # ---- softmax over batch (local: all 32 batches on this core) ----
            # att rows are laid p = chunk*32 + b, cols = s within chunk
            th = finalp.tile([P, 512], F32, tag="th", name="th")
            nc.scalar.activation(th[:], att_ps[:], AF.Tanh)
            ex = finalp.tile([P, 512], DT, tag="ex", name="ex")
            nc.scalar.activation(ex[:], th[:], AF.Exp, scale=10.0)

            den_ps = ps_misc.tile([NCHUNK, 512], F32, tag="ps_misc", name="den_ps")
            nc.tensor.matmul(
                den_ps[:], lhsT=sel_sb[:], rhs=ex[:], start=True, stop=True
            )

            # den_ps holds den/4096 (sel = 2^-12), so inv = 4096/den sits in
            # fp16 normal range ([9.5e-3, 7.8e-2] on the real data; raw 1/den
            # would be subnormal). selb = 2^-12 restores 1/den exactly in the
            # f32 psum of the broadcast matmul.
            inv = finalp.tile([NCHUNK, 512], DT, tag="inv", name="inv")
            with nc.allow_low_precision(reason="scaled fp16 softmax inv; 2e-2 tol"):
                nc.vector.reciprocal(inv[:], den_ps[:])

            # broadcast inv [4, 512] -> [128, 512] via a K=4 select matmul:
            # invrep[p, s] = inv[p//32, s]
            invrep_ps = ps_tanh.tile([P, 512], F32, tag="pt", name="invrep_ps")
            nc.tensor.matmul(
                invrep_ps[:], lhsT=selb_sb[:], rhs=inv[:], start=True, stop=True
            )

            outv = finalp.tile([P, 512], F32, tag="outv", name="outv")
            nc.vector.tensor_mul(outv[:], ex[:], invrep_ps[:])

            # 4 plain DMAs (one per chunk), split across the two HWDGE rings
            # (SP and ACT — the scalar engine is idle by this point) so the
            # tail transfers run in parallel. A single merged DMA with a
            # dram-side rearrange costs +22us in the DMA model (permuted
            # descriptor stream) — keep 4 simple ones.
            for c in range(NCHUNK):
                eng = nc.sync if c % 2 == 0 else nc.scalar
                eng.dma_start(
                    out=out[:, c * 512 : (c + 1) * 512],
                    in_=outv[c * 32 : (c + 1) * 32, :],
                )

        if loop_n:
            import concourse.mybir as _mb

            with tc.For_i(
                0,
                loop_n,
                1,
                hint_engines=(
                    _mb.EngineType.PE,
                    _mb.EngineType.Activation,
                    _mb.EngineType.DVE,
                    _mb.EngineType.SP,
                    _mb.EngineType.Pool,
                ),
                staggered_reset=True,
            ):
                emit_body()
        else:
            for _rep in range(repeat):
                emit_body()

    nc.compile()
    return nc


def _host_prep(inputs, np_dt):
    """Build the per-core input maps from the full problem inputs."""
    x = np.asarray(inputs["x"], np.float32)
    context = np.asarray(inputs["context"], np.float32)
    W_in = np.asarray(inputs["W_in"], np.float32)
    b_in = np.asarray(inputs["b_in"], np.float32)
    Wc = np.stack(
        [np.asarray(inputs[f"Wc{i}"], np.float32) for i in range(NBR)]
    )  # [br, h, d]
    bc = np.stack([np.asarray(inputs[f"bc{i}"], np.float32) for i in range(NBR)])
    V = np.stack([np.asarray(inputs[f"V{i}"], np.float32) for i in range(NBR)])

    # ctxT_full[b, dt, p, s] = context[b, s, dt*128+p] in low precision
    ctxT_full = np.ascontiguousarray(context.transpose(0, 2, 1)).astype(
        np_dt
    )  # [B, D, S]
    ctxT_full = ctxT_full.reshape(B, DTILES, P, S)

    # ones window: col 127 all-ones; slice [127-j : 255-j] puts the ones
    # column at local position j (routes partition-sums to output row j)
    ones_w = np.zeros((P, 256), np_dt)
    ones_w[:, 127] = 1.0

    # winT[p, dt, ht, j] = W_in[ht*128+j, dt*128+p]
    winT = np.ascontiguousarray(
        W_in.reshape(HT, P, DTILES, P).transpose(3, 2, 0, 1)
    ).astype(np_dt)

    # xT[p, dt, b] = x[b, dt*128+p]
    xT = np.ascontiguousarray(x.reshape(B, DTILES, P).transpose(2, 1, 0)).astype(np_dt)

    # sel[p, m] = 1 if p//32 == m
    # den matmul weights carry a 2^-12 scale so the fp16 reciprocal of
    # the (scaled) denominator stays in normal range; selb carries the
    # matching 2^-12 so the broadcast matmul restores exactly 1/den.
    sel = np.zeros((P, NCHUNK), np_dt)
    for m in range(NCHUNK):
        sel[m * 32 : (m + 1) * 32, m] = 2.0 ** -12
    selb = np.zeros((NCHUNK, P), np_dt)
    for m in range(NCHUNK):
        selb[m, m * 32 : (m + 1) * 32] = 2.0 ** -12

    in_maps = []
    for k in range(NCORES):
        br, shalf = k >> 1, k & 1
        # wcT[p, dt, ht, j] = Wc[br, ht*128+j, dt*128+p]
        wcT = np.ascontiguousarray(
            Wc[br].reshape(HT, P, DTILES, P).transpose(3, 2, 0, 1)
        ).astype(np_dt)
        # vcols[p, ht] = V[br, ht*128+p]
        vcols = np.ascontiguousarray(
            V[br].reshape(HT, P).transpose(1, 0)
        ).astype(np.float32)
        # bcomb[p, ht] = b_in[ht*128+p] + bc[br, ht*128+p]
        bsum = b_in + bc[br]  # [H]
        bcomb_k = np.ascontiguousarray(
            bsum.reshape(HT, P).transpose(1, 0)
        ).astype(np.float32)
        ctx_k = np.ascontiguousarray(
            ctxT_full[:, :, :, shalf * SH : (shalf + 1) * SH]
        )
        in_maps.append(
            dict(
                ctxT=ctx_k,
                wcT=wcT,
                ones_w=ones_w,
                vcols=vcols,
                winT=winT,
                xT=xT,
                bcomb=bcomb_k,
                sel=sel,
                selb=selb,
            )
        )
    return in_maps


def kernel(**inputs) -> np.ndarray:
    dt_name = os.environ.get("KERNEL_DT", "float16")
    np_dt = {"float16": np.float16, "bfloat16": None}[dt_name]
    if np_dt is None:
        import ml_dtypes

        np_dt = ml_dtypes.bfloat16

    from concourse import bass_utils

    if dt_name not in _CACHE:
        _CACHE[dt_name] = _build_nc(dt_name)
    nc = _CACHE[dt_name]

    in_maps = _host_prep(inputs, np_dt)
    res = bass_utils.run_bass_kernel_spmd(nc, in_maps, core_ids=list(range(NCORES)))

    # core k=(br, shalf) produced out[b, s] for att columns
    # [br*S + shalf*SH, +SH)
    full = np.empty((B, NBR, 2, SH), np.float32)
    for k in range(NCORES):
        br, shalf = k >> 1, k & 1
        full[:, br, shalf, :] = res.results[k]["out"]
    return full.reshape(B, NBR * S).astype(np.float32)


if __name__ == "__main__":
    # smoke test with random inputs
    rng = np.random.default_rng(0)
    inputs = dict(
        x=rng.standard_normal((B, H), dtype=np.float32),
        context=rng.standard_normal((B, S, D), dtype=np.float32),
        mask=np.ones((B, S), bool),
        W_in=rng.uniform(-1 / 16, 1 / 16, (H, H)).astype(np.float32),
        b_in=rng.uniform(-1 / 16, 1 / 16, (H,)).astype(np.float32),
    )
    for i in range(4):
        inputs[f"Wc{i}"] = rng.uniform(-1 / 16, 1 / 16, (H, D)).astype(np.float32)
        inputs[f"bc{i}"] = rng.uniform(-1 / 16, 1 / 16, (H,)).astype(np.float32)
        inputs[f"V{i}"] = rng.uniform(-1, 1, (H,)).astype(np.float32)
    out = kernel(**inputs)
    print("out", out.shape, out.dtype, out.sum())


# revision 32
# speedup vs baseline: 1.0517x; 1.0517x over previous
"""Trainium2 Bass kernel for nn_Attention_57432302682539.

Reference computation (B=32, S=4096, D=256, H=256):
    inp = x @ W_in.T + b_in                                  # [B, H]
    branch_i: ctx = einsum('bsd,hd->bhs', context, Wc_i) + bc_i
              att_i = einsum('h,bhs->bs', V_i, tanh(inp[:,:,None] + ctx))
    att = concat(att_0..3, axis=1)                           # [B, 4S]
    att = 10*tanh(att)  (mask is all ones -> where() is identity)
    out = softmax(att, axis=0)                               # over batch

Sharding: BRANCH x S-HALF.  Core c handles branch br=c>>1 and s-columns
[sh*2048, sh*2048+2048) with sh=c&1, for ALL 32 batches.  The dim-0
(batch) softmax acts per output column (br, s), so with every batch
resident the softmax is entirely core-local -- no collective.

Compared to the previous S-only shard this grows the per-(b,ht) tanh
tile from [128,512] to [128,2048], quartering the ACT instruction count
(the scalar engine, at 1 elem/cycle/lane with ~200ns/instr overhead, is
the bottleneck engine together with PE).

Per-core pipeline, per batch b (32 iterations):
  - DMA ctxT[b] [2dt,128,2048] fp16 (prefetched, triple-buffered)
  - PE: wcT.T @ ctx -> PSUM units [128h,1024s] (K=256 via 2 accum steps)
  - ACT: tanh with per-partition bias (inp[b]+bc) -> SBUF fp16 T tiles
  - DVE: s_t = V_ht0*T0 + V_ht1*T1  (per-partition scalars)
  - PE: ones-window matmul routes partition-sum of s_t chunk c into row
    c*32+b of a single persistent PSUM accumulator [128, 512]
  - final: exp(10*tanh(att)) on ACT (fp16), batch-group sums via a
    2^-12-scaled fp16 select matmul, fp16 DVE reciprocal (the scale keeps
    it in fp16 normal range), K=4 select matmul to broadcast 1/den back
    to 128 partitions, DVE mul, out-DMAs split across both HWDGE rings.

PSUM budget (8 banks): 3 x 2-bank tanh units + 1 att accumulator +
1 misc (inp matmul, then softmax denominator).
"""

import os
import numpy as np

B, S, D, H = 32, 4096, 256, 256
NCORES = 8
NBR = 4                   # branches
SH = S // 2               # 2048 s-positions per core (one half)
HT = 2                    # h tiles of 128
DTILES = 2                # d tiles of 128
P = 128
NCHUNK = 4                # 512-col chunks of SH; att row = chunk*32 + b

_CACHE = {}


def _build_nc(dt_name: str, repeat: int = 1, loop_n: int = 0, variant: str = "full"):
    """Build the Bass module. dt_name in ('float16', 'bfloat16').

    repeat>1 unrolls the whole computation N times inside the NEFF; loop_n>0
    instead wraps it in a hardware For_i loop with that trip count (both for
    on-device timing via wall-clock differencing); the result is unchanged.
    """
    import concourse.bass as bass
    import concourse.mybir as mybir
    import concourse.tile as tile
    from concourse import bacc

    DT = getattr(mybir.dt, dt_name)
    F32 = mybir.dt.float32
    AF = mybir.ActivationFunctionType

    nc = bacc.Bacc(
        trn_type="TRN2",
        use_seq_codegen=os.environ.get("KERNEL_SEQ_CODEGEN", "0") == "1",
    )

    # Per-core external inputs (host-preprocessed, branch/s-half specific).
    ctxT = nc.dram_tensor("ctxT", [B, DTILES, P, SH], DT, kind="ExternalInput")
    wcT = nc.dram_tensor("wcT", [P, DTILES, HT, P], DT, kind="ExternalInput")
    ones_w = nc.dram_tensor("ones_w", [P, 256], DT, kind="ExternalInput")
    vcols = nc.dram_tensor("vcols", [P, HT], F32, kind="ExternalInput")
    winT = nc.dram_tensor("winT", [P, DTILES, HT, P], DT, kind="ExternalInput")
    xT = nc.dram_tensor("xT", [P, DTILES, B], DT, kind="ExternalInput")
    bcomb = nc.dram_tensor("bcomb", [P, HT], F32, kind="ExternalInput")
    sel = nc.dram_tensor("sel", [P, NCHUNK], DT, kind="ExternalInput")
    selb = nc.dram_tensor("selb", [NCHUNK, P], DT, kind="ExternalInput")
    out = nc.dram_tensor("out", [B, SH], F32, kind="ExternalOutput")

    from contextlib import ExitStack

    with tile.TileContext(nc) as tc, ExitStack() as ctx:
        const = ctx.enter_context(tc.tile_pool(name="const", bufs=1))
        ctxp = ctx.enter_context(
            tc.tile_pool(
                name="ctxp", bufs=int(os.environ.get("KERNEL_PF", "6")) + 1
            )
        )
        tanhp = ctx.enter_context(tc.tile_pool(name="tanhp", bufs=10))
        stp = ctx.enter_context(tc.tile_pool(name="stp", bufs=6))
        finalp = ctx.enter_context(tc.tile_pool(name="finalp", bufs=2))
        ps_tanh = ctx.enter_context(tc.tile_pool(name="ps_tanh", bufs=3, space="PSUM"))
        ps_att = ctx.enter_context(tc.tile_pool(name="ps_att", bufs=1, space="PSUM"))
        ps_misc = ctx.enter_context(tc.tile_pool(name="ps_misc", bufs=1, space="PSUM"))

        # ---- constants into SBUF ----
        wcT_sb = const.tile([P, DTILES, HT, P], DT)
        nc.sync.dma_start(out=wcT_sb, in_=wcT[:])
        ones_sb = const.tile([P, 256], DT)
        nc.sync.dma_start(out=ones_sb, in_=ones_w[:])
        vcols_sb = const.tile([P, HT], F32)
        nc.sync.dma_start(out=vcols_sb, in_=vcols[:])
        winT_sb = const.tile([P, DTILES, HT, P], DT)
        nc.sync.dma_start(out=winT_sb, in_=winT[:])
        xT_sb = const.tile([P, DTILES, B], DT)
        nc.sync.dma_start(out=xT_sb, in_=xT[:])
        bcomb_sb = const.tile([P, HT], F32)
        nc.sync.dma_start(out=bcomb_sb, in_=bcomb[:])
        sel_sb = const.tile([P, NCHUNK], DT)
        nc.sync.dma_start(out=sel_sb, in_=sel[:])
        selb_sb = const.tile([NCHUNK, P], DT)
        nc.sync.dma_start(out=selb_sb, in_=selb[:])

        do_act = variant not in ("mm_only",)
        do_dve = variant not in ("mm_only", "act_only")
        do_vsum = variant not in ("mm_only", "act_only", "no_vsum")
        do_final = do_vsum

        def emit_body():
            # ---- inp = x @ W_in.T (+ b_in + bc), h on partitions ----
            # bias_all[:, ht, b] = sum_d W_in[h,d] x[b,d] + b_in[h] + bc[h]
            bias_all = const.tile([P, HT, B], F32, tag="bias", name="bias_all")
            for ht in range(HT):
                ps_inp = ps_misc.tile([P, B], F32, tag="ps_misc", name="ps_inp")
                for dti in range(DTILES):
                    nc.tensor.matmul(
                        ps_inp[:],
                        lhsT=winT_sb[:, dti, ht],
                        rhs=xT_sb[:, dti],
                        start=(dti == 0),
                        stop=(dti == DTILES - 1),
                    )
                nc.scalar.add(bias_all[:, ht], ps_inp[:], bcomb_sb[:, ht : ht + 1])

            att_ps = ps_att.tile([P, 512], F32, tag="att", name="att_ps")

            # lagged work queues
            pend_dve = []   # (T0_half, T1_half, b, half)
            pend_mm = []    # (s_t half tile, b, half)
            dve_idx = [0]
            mm_idx = [0]
            n_halves = B * 2
            # halves between ACT and the DVE V-weighting / further halves
            # between DVE and the PE partition-sum
            DVE_LAG = int(os.environ.get("KERNEL_DVE_LAG", "2"))
            MM_LAG = int(os.environ.get("KERNEL_MM_LAG", "4"))

            def emit_dve():
                ip = dve_idx[0]
                T0, T1, b, half = pend_dve[ip]
                s_t = stp.tile([P, 1024], DT, tag="acc", name="s_t")
                if ip == n_halves - 1:
                    # last half sits on the serial tail: combine in two
                    # 512-col pieces so the first vsum matmul (and the T0
                    # mul) can start before the final tanh/add complete
                    for pc in range(2):
                        sl = slice(pc * 512, (pc + 1) * 512)
                        u_s = stp.tile([P, 512], DT, tag="accs", name="u_s")
                        nc.vector.tensor_scalar_mul(
                            u_s[:], T0[:, sl], vcols_sb[:, 0:1]
                        )
                        w_s = stp.tile([P, 512], DT, tag="accs", name="w_s")
                        nc.vector.tensor_scalar_mul(
                            w_s[:], T1[:, sl], vcols_sb[:, 1:2]
                        )
                        nc.vector.tensor_add(s_t[:, sl], u_s[:], w_s[:])
                else:
                    u_t = stp.tile([P, 1024], DT, tag="acc", name="u_t")
                    nc.vector.tensor_scalar_mul(u_t[:], T0[:], vcols_sb[:, 0:1])
                    w_t = stp.tile([P, 1024], DT, tag="acc", name="w_t")
                    nc.vector.tensor_scalar_mul(w_t[:], T1[:], vcols_sb[:, 1:2])
                    nc.vector.tensor_add(s_t[:], u_t[:], w_t[:])
                pend_mm.append((s_t, b, half))
                dve_idx[0] += 1

            def emit_vsum():
                ip = mm_idx[0]
                s_t, b, half = pend_mm[ip]
                for uc in range(2):
                    c = half * 2 + uc
                    j = c * 32 + b
                    nc.tensor.matmul(
                        att_ps[:],
                        lhsT=ones_sb[:, 127 - j : 255 - j],
                        rhs=s_t[:, uc * 512 : (uc + 1) * 512],
                        start=(ip == 0 and uc == 0),
                        stop=(ip == n_halves - 1 and uc == 1),
                    )
                mm_idx[0] += 1

            PF = int(os.environ.get("KERNEL_PF", "6"))  # ctx DMA prefetch depth
            ctx_tiles = [None] * B

            def load_ctx(b):
                tp = ctxp.tile([P, DTILES, SH], DT, tag="ctx", name="ctxb")
                src = ctxT[b].rearrange("dt p s -> p dt s")
                nc.sync.dma_start(out=tp, in_=src)
                ctx_tiles[b] = tp

            # b=0 lands as two separate tiles (512 + 1536 s-cols) so the
            # first fill matmuls start after ~0.8us instead of ~3.2us;
            # chunk reads below are 512-aligned so the split is transparent.
            ctx0_parts = [None, None]

            def load_ctx0_split():
                pa = ctxp.tile([P, DTILES, 512], DT, tag="ctx0a", name="ctx0a")
                nc.sync.dma_start(
                    out=pa, in_=ctxT[0, :, :, 0:512].rearrange("dt p s -> p dt s")
                )
                pb = ctxp.tile([P, DTILES, SH - 512], DT, tag="ctx0b", name="ctx0b")
                nc.sync.dma_start(
                    out=pb, in_=ctxT[0, :, :, 512:SH].rearrange("dt p s -> p dt s")
                )
                ctx0_parts[0], ctx0_parts[1] = pa, pb

            def ctx_chunk(b, dti, s0):
                # [128, 512] chunk of batch b's context at s-offset s0
                if b == 0 or (variant == "no_dma"):
                    if s0 == 0:
                        return ctx0_parts[0][:, dti, 0:512]
                    return ctx0_parts[1][:, dti, s0 - 512 : s0]
                return ctx_tiles[b][:, dti, s0 : s0 + 512]

            if variant == "no_dma":
                load_ctx0_split()
            else:
                load_ctx0_split()
                for b in range(1, min(PF, B)):
                    load_ctx(b)

            for b in range(B):
                if variant != "no_dma" and b + PF < B:
                    load_ctx(b + PF)
                T_halves = [[None, None], [None, None]]  # [half][ht]
                for half in range(2):
                    for ht in range(HT):
                        ut = ps_tanh.tile([P, 1024], F32, tag="pt", name="ut")
                        for uc in range(2):
                            s0 = half * 1024 + uc * 512
                            for dti in range(DTILES):
                                nc.tensor.matmul(
                                    ut[:, uc * 512 : (uc + 1) * 512],
                                    lhsT=wcT_sb[:, dti, ht],
                                    rhs=ctx_chunk(b, dti, s0),
                                    start=(dti == 0),
                                    stop=(dti == DTILES - 1),
                                )
                        if do_act:
                            tt = tanhp.tile([P, 1024], DT, tag="tanh", name="tt")
                            nc.scalar.activation(
                                tt[:], ut[:], AF.Tanh, bias=bias_all[:, ht, b : b + 1]
                            )
                            T_halves[half][ht] = tt
                    if do_dve:
                        pend_dve.append(
                            (T_halves[half][0], T_halves[half][1], b, half)
                        )
                        ip = len(pend_dve) - 1
                        if ip >= DVE_LAG:
                            emit_dve()
                        if do_vsum and ip >= DVE_LAG + MM_LAG:
                            emit_vsum()
                if variant != "no_dma":
                    ctx_tiles[b] = None

            if do_dve:
                while dve_idx[0] < n_halves:
                    emit_dve()
            if do_vsum:
                while mm_idx[0] < n_halves:
                    emit_vsum()

            if not do_final:
                return

            # ---- softmax over batch (local: all 32 batches on this core) ----
            # att rows are laid p = chunk*32 + b, cols = s within chunk
            th = finalp.tile([P, 512], F32, tag="th", name="th")
            nc.scalar.activation(th[:], att_ps[:], AF.Tanh)
            ex = finalp.tile([P, 512], DT, tag="ex", name="ex")
            nc.scalar.activation(ex[:], th[:], AF.Exp, scale=10.0)

            den_ps = ps_misc.tile([NCHUNK, 512], F32, tag="ps_misc", name="den_ps")
            nc.tensor.matmul(
                den_ps[:], lhsT=sel_sb[:], rhs=ex[:], start=True, stop=True
            )

            # den_ps holds den/4096 (sel = 2^-12), so inv = 4096/den sits in
            # fp16 normal range ([9.5e-3, 7.8e-2] on the real data; raw 1/den
            # would be subnormal). selb = 2^-12 restores 1/den exactly in the
            # f32 psum of the broadcast matmul.
            inv = finalp.tile([NCHUNK, 512], DT, tag="inv", name="inv")
            with nc.allow_low_precision(reason="scaled fp16 softmax inv; 2e-2 tol"):
                nc.vector.reciprocal(inv[:], den_ps[:])

            # broadcast inv [4, 512] -> [128, 512] via a K=4 select matmul:
            # invrep[p, s] = inv[p//32, s]
            invrep_ps = ps_tanh.tile([P, 512], F32, tag="pt", name="invrep_ps")
            nc.tensor.matmul(
                invrep_ps[:], lhsT=selb_sb[:], rhs=inv[:], start=True, stop=True
            )

            outv = finalp.tile([P, 512], F32, tag="outv", name="outv")
            nc.vector.tensor_mul(outv[:], ex[:], invrep_ps[:])

            # 4 plain DMAs (one per chunk), split across the two HWDGE rings
            # (SP and ACT — the scalar engine is idle by this point) so the
            # tail transfers run in parallel. A single merged DMA with a
            # dram-side rearrange costs +22us in the DMA model (permuted
            # descriptor stream) — keep 4 simple ones.
            for c in range(NCHUNK):
                eng = nc.sync if c % 2 == 0 else nc.scalar
                eng.dma_start(
                    out=out[:, c * 512 : (c + 1) * 512],
                    in_=outv[c * 32 : (c + 1) * 32, :],
                )

        if loop_n:
            import concourse.mybir as _mb

            with tc.For_i(
                0,
                loop_n,
                1,
                hint_engines=(
                    _mb.EngineType.PE,
                    _mb.EngineType.Activation,
                    _mb.EngineType.DVE,
                    _mb.EngineType.SP,
                    _mb.EngineType.Pool,
                ),
                staggered_reset=True,
            ):
                emit_body()
        else:
            for _rep in range(repeat):
                emit_body()

    nc.compile()
    return nc


def _host_prep(inputs, np_dt):
    """Build the per-core input maps from the full problem inputs."""
    x = np.asarray(inputs["x"], np.float32)
    context = np.asarray(inputs["context"], np.float32)
    W_in = np.asarray(inputs["W_in"], np.float32)
    b_in = np.asarray(inputs["b_in"], np.float32)
    Wc = np.stack(
        [np.asarray(inputs[f"Wc{i}"], np.float32) for i in range(NBR)]
    )  # [br, h, d]
    bc = np.stack([np.asarray(inputs[f"bc{i}"], np.float32) for i in range(NBR)])
    V = np.stack([np.asarray(inputs[f"V{i}"], np.float32) for i in range(NBR)])

    # ctxT_full[b, dt, p, s] = context[b, s, dt*128+p] in low precision
    ctxT_full = np.ascontiguousarray(context.transpose(0, 2, 1)).astype(
        np_dt
    )  # [B, D, S]
    ctxT_full = ctxT_full.reshape(B, DTILES, P, S)

    # ones window: col 127 all-ones; slice [127-j : 255-j] puts the ones
    # column at local position j (routes partition-sums to output row j)
    ones_w = np.zeros((P, 256), np_dt)
    ones_w[:, 127] = 1.0

    # winT[p, dt, ht, j] = W_in[ht*128+j, dt*128+p]
    winT = np.ascontiguousarray(
        W_in.reshape(HT, P, DTILES, P).transpose(3, 2, 0, 1)
    ).astype(np_dt)

    # xT[p, dt, b] = x[b, dt*128+p]
    xT = np.ascontiguousarray(x.reshape(B, DTILES, P).transpose(2, 1, 0)).astype(np_dt)

    # sel[p, m] = 1 if p//32 == m
    # den matmul weights carry a 2^-12 scale so the fp16 reciprocal of
    # the (scaled) denominator stays in normal range; selb carries the
    # matching 2^-12 so the broadcast matmul restores exactly 1/den.
    sel = np.zeros((P, NCHUNK), np_dt)
    for m in range(NCHUNK):
        sel[m * 32 : (m + 1) * 32, m] = 2.0 ** -12
    selb = np.zeros((NCHUNK, P), np_dt)
    for m in range(NCHUNK):
        selb[m, m * 32 : (m + 1) * 32] = 2.0 ** -12

    in_maps = []
    for k in range(NCORES):
        br, shalf = k >> 1, k & 1
        # wcT[p, dt, ht, j] = Wc[br, ht*128+j, dt*128+p]
        wcT = np.ascontiguousarray(
            Wc[br].reshape(HT, P, DTILES, P).transpose(3, 2, 0, 1)
        ).astype(np_dt)
        # vcols[p, ht] = V[br, ht*128+p]
        vcols = np.ascontiguousarray(
            V[br].reshape(HT, P).transpose(1, 0)
        ).astype(np.float32)
        # bcomb[p, ht] = b_in[ht*128+p] + bc[br, ht*128+p]
        bsum = b_in + bc[br]  # [H]
        bcomb_k = np.ascontiguousarray(
            bsum.reshape(HT, P).transpose(1, 0)
        ).astype(np.float32)
        ctx_k = np.ascontiguousarray(
            ctxT_full[:, :, :, shalf * SH : (shalf + 1) * SH]
        )
        in_maps.append(
            dict(
                ctxT=ctx_k,
                wcT=wcT,
                ones_w=ones_w,
                vcols=vcols,
                winT=winT,
                xT=xT,
                bcomb=bcomb_k,
                sel=sel,
                selb=selb,
            )
        )
    return in_maps


def kernel(**inputs) -> np.ndarray:
    dt_name = os.environ.get("KERNEL_DT", "float16")
    np_dt = {"float16": np.float16, "bfloat16": None}[dt_name]
    if np_dt is None:
        import ml_dtypes

        np_dt = ml_dtypes.bfloat16

    from concourse import bass_utils

    if dt_name not in _CACHE:
        _CACHE[dt_name] = _build_nc(dt_name)
    nc = _CACHE[dt_name]

    in_maps = _host_prep(inputs, np_dt)
    res = bass_utils.run_bass_kernel_spmd(nc, in_maps, core_ids=list(range(NCORES)))

    # core k=(br, shalf) produced out[b, s] for att columns
    # [br*S + shalf*SH, +SH)
    full = np.empty((B, NBR, 2, SH), np.float32)
    for k in range(NCORES):
        br, shalf = k >> 1, k & 1
        full[:, br, shalf, :] = res.results[k]["out"]
    return full.reshape(B, NBR * S).astype(np.float32)


if __name__ == "__main__":
    # smoke test with random inputs
    rng = np.random.default_rng(0)
    inputs = dict(
        x=rng.standard_normal((B, H), dtype=np.float32),
        context=rng.standard_normal((B, S, D), dtype=np.float32),
        mask=np.ones((B, S), bool),
        W_in=rng.uniform(-1 / 16, 1 / 16, (H, H)).astype(np.float32),
        b_in=rng.uniform(-1 / 16, 1 / 16, (H,)).astype(np.float32),
    )
    for i in range(4):
        inputs[f"Wc{i}"] = rng.uniform(-1 / 16, 1 / 16, (H, D)).astype(np.float32)
        inputs[f"bc{i}"] = rng.uniform(-1 / 16, 1 / 16, (H,)).astype(np.float32)
        inputs[f"V{i}"] = rng.uniform(-1, 1, (H,)).astype(np.float32)
    out = kernel(**inputs)
    print("out", out.shape, out.dtype, out.sum())
